# revision 1
# baseline (speedup 1.0000x reference)
"""Edge-parallel GNN message-passing MLP on 8 TRN2 NeuronCores.

Computation (per edge e): out[e] = relu(concat(x[row[e]], edge_attr[e]) @ W1 + b1) @ W2 + b2

Sharding: edges split evenly across the 8 cores (edge-parallel); x and the MLP
weights are replicated. Per core, per 2048-edge tile:
  - dma_gather fetches the x row-pair x2[row>>1] (512 B) for each edge
    (row-pair indexing keeps the gather indices within int16 range)
  - a parity select keeps the correct 256 B half; edge_attr is DMAed into the
    other half of the same edge-major tile
  - PE transposes 128x128 blocks to feature-major, then a 2-layer MLP runs in
    fp32r (full-rate fp32) with relu+bias fused on the scalar engine
  - results stream back as one contiguous 1 MiB store per tile

Tile edge mapping is partition-minor (edge = block*128 + partition) to match
dma_gather's output layout.

Self-contained: shapes/sharding are hardcoded for the 50000-node / 800000-edge
/ 64-feature problem instance.
"""

from contextlib import ExitStack

import numpy as np

import concourse.bacc as bacc_mod
import concourse.bass as bass
import concourse.mybir as mybir
import concourse.tile as tile
from concourse.bass_utils import run_bass_kernel_spmd
from concourse.masks import make_identity

N_CORES = 8
N_NODES = 50000
N_EDGES = 800000
F_IN = 64
HIDDEN = 128
F_OUT = 128

E_REAL = N_EDGES // N_CORES  # 100000 edges per core
TILE_E = 2048                # edges per pipeline tile
NT = 49                      # tiles per core
EPC = NT * TILE_E            # 100352 padded edges per core
KPT = TILE_E // 128          # 16 128-edge blocks per tile
QUARTER = 4                  # 128-edge blocks per PSUM-stage quarter

F32 = mybir.dt.float32
F32R = mybir.dt.float32r
I16 = mybir.dt.int16
I8 = mybir.dt.int8

RELU = mybir.ActivationFunctionType.Relu
ADD = mybir.AluOpType.add


def build_program(nt: int = NT):
    epc = nt * TILE_E
    nc = bacc_mod.Bacc("TRN2")

    # x viewed as row pairs: x2[i] = concat(x[2i], x[2i+1])
    x2_d = nc.declare_dram_parameter("x2", [N_NODES // 2, 2 * F_IN], F32, isOutput=False)
    # gather indices (row>>1) in dma_gather's [16, n/16] wrap, tiled to 128 partitions
    hidx_d = nc.declare_dram_parameter("hidx", [nt * 128, TILE_E // 16], I16, isOutput=False)
    # row parity as f32 mask, [tile, partition, block] layout
    par_d = nc.declare_dram_parameter("par", [nt * 128, KPT], I8, isOutput=False)
    ea_d = nc.declare_dram_parameter("ea", [epc, F_IN], F32, isOutput=False)
    w1_d = nc.declare_dram_parameter("w1", [2 * F_IN, HIDDEN], F32, isOutput=False)
    w2p_d = nc.declare_dram_parameter("w2p", [HIDDEN, 2 * F_OUT], F32, isOutput=False)
    b1_d = nc.declare_dram_parameter("b1c", [HIDDEN, 1], F32, isOutput=False)
    b2_d = nc.declare_dram_parameter("b2", [F_OUT], F32, isOutput=False)
    out_d = nc.declare_dram_parameter("out", [epc, F_OUT], F32, isOutput=True)

    # edge e = t*TILE_E + c*128 + p  <->  (tile t, partition p, block c)
    hidx_r = hidx_d[:, :].rearrange("(t p) s -> t p s", p=128)
    par_r = par_d[:, :].rearrange("(t p) c -> t p c", p=128)
    ea_r = ea_d[:, :].rearrange("(t c p) f -> t p c f", c=KPT, p=128)
    out_r = out_d[:, :].rearrange("(t c p) f -> t p c f", c=KPT, p=128)

    with tile.TileContext(nc) as tc, ExitStack() as ctx:
        const = ctx.enter_context(tc.tile_pool(name="const", bufs=1))
        idx_p = ctx.enter_context(tc.tile_pool(name="idx", bufs=2))
        xg2_p = ctx.enter_context(tc.tile_pool(name="xg2", bufs=2))
        feats_p = ctx.enter_context(tc.tile_pool(name="feats", bufs=2))
        ftsb_p = ctx.enter_context(tc.tile_pool(name="ftsb", bufs=2))
        h1sb_p = ctx.enter_context(tc.tile_pool(name="h1sb", bufs=2))
        outsb_p = ctx.enter_context(tc.tile_pool(name="outsb", bufs=2))
        ftps_p = ctx.enter_context(tc.tile_pool(name="ftps", bufs=2, space="PSUM"))
        h1ps_p = ctx.enter_context(tc.tile_pool(name="h1ps", bufs=2, space="PSUM"))
        outps_p = ctx.enter_context(tc.tile_pool(name="outps", bufs=2, space="PSUM"))

        # ---- constants (loaded once) ----
        w1_raw = const.tile([128, HIDDEN], F32, tag="w1_raw")
        nc.sync.dma_start(out=w1_raw, in_=w1_d[:, :])
        w1_t = const.tile([128, HIDDEN], F32R, tag="w1")
        nc.vector.tensor_copy(out=w1_t, in_=w1_raw)
        w2p_raw = const.tile([128, 2 * F_OUT], F32, tag="w2p_raw")
        nc.sync.dma_start(out=w2p_raw, in_=w2p_d[:, :])
        w2p_t = const.tile([128, 2 * F_OUT], F32R, tag="w2p")
        nc.vector.tensor_copy(out=w2p_t, in_=w2p_raw)
        b1_t = const.tile([128, 1], F32, tag="b1")
        nc.sync.dma_start(out=b1_t, in_=b1_d[:, :])
        # b2 replicated: [128 partitions, 4 blocks, 128] all copies of b2
        b2f_t = const.tile([128, QUARTER, F_OUT], F32, tag="b2f")
        b2_ap = b2_d[:]
        b2_bcast = bass.AP(b2_ap.tensor, b2_ap.offset, [[0, 128], [0, QUARTER], [1, F_OUT]])
        nc.gpsimd.dma_start(out=b2f_t, in_=b2_bcast)
        ident = const.tile([128, 128], F32, tag="ident")
        make_identity(nc, ident)

        for t in range(nt):
            # ---- load gather indices + parity mask ----
            idx16 = idx_p.tile([128, TILE_E // 16], I16, tag="idx16")
            nc.sync.dma_start(out=idx16, in_=hidx_r[t])
            part = idx_p.tile([128, KPT, 1], I8, tag="par")
            nc.sync.dma_start(out=part[:, :, 0], in_=par_r[t])

            # ---- gather x row pairs ----
            xg2 = xg2_p.tile([128, KPT, 2 * F_IN], F32, tag="xg2")
            nc.gpsimd.dma_gather(
                xg2[:, :, :],
                x2_d[:, :],
                idx16[:, :],
                TILE_E,
                TILE_E,
                2 * F_IN,
                single_packet=False,
            )

            # ---- build edge-major feats tile: [x_selected | edge_attr] ----
            feats = feats_p.tile([128, KPT, 2 * F_IN], F32, tag="feats")
            nc.scalar.copy(out=feats[:, :, 0:F_IN], in_=xg2[:, :, 0:F_IN])
            nc.vector.copy_predicated(
                out=feats[:, :, 0:F_IN],
                mask=part.to_broadcast([128, KPT, F_IN]),
                data=xg2[:, :, F_IN : 2 * F_IN],
            )
            nc.sync.dma_start(out=feats[:, :, F_IN : 2 * F_IN], in_=ea_r[t])

            h1sb = h1sb_p.tile([128, KPT, HIDDEN], F32R, tag="h1sb")
            out_sb = outsb_p.tile([128, KPT, F_OUT], F32, tag="out_sb")

            for q in range(KPT // QUARTER):
                # ---- transpose 4x [128 edges, 128 feats] -> [128 feats, 512 edges] ----
                ftps = ftps_p.tile([128, QUARTER * 128], F32, tag="ftps", space="PSUM")
                for j in range(QUARTER):
                    nc.tensor.transpose(
                        out=ftps[:, j * 128 : (j + 1) * 128],
                        in_=feats[:, q * QUARTER + j, :],
                        identity=ident,
                    )
                ftsb = ftsb_p.tile([128, QUARTER * 128], F32R, tag="ftsb")
                nc.vector.tensor_copy(out=ftsb, in_=ftps)

                # ---- layer 1: h1T[H, 512] = W1.T @ featsT ----
                h1ps = h1ps_p.tile([128, QUARTER * 128], F32, tag="h1ps", space="PSUM")
                nc.tensor.matmul(
                    out=h1ps,
                    lhsT=w1_t,
                    rhs=ftsb,
                    start=True,
                    stop=True,
                )
                nc.scalar.activation(
                    out=h1sb[:, q * QUARTER : (q + 1) * QUARTER, :],
                    in_=h1ps.rearrange("h (a b) -> h a b", a=QUARTER),
                    func=RELU,
                    bias=b1_t,
                    scale=1.0,
                )

                # ---- layer 2: out[128 edges, 256] = h1T_k.T @ W2pad ----
                outps = outps_p.tile([128, QUARTER, 2 * F_OUT], F32, tag="outps", space="PSUM")
                for j in range(QUARTER):
                    nc.tensor.matmul(
                        out=outps[:, j, :],
                        lhsT=h1sb[:, q * QUARTER + j, :],
                        rhs=w2p_t,
                        start=True,
                        stop=True,
                    )
                nc.vector.tensor_tensor(
                    out=out_sb[:, q * QUARTER : (q + 1) * QUARTER, :],
                    in0=outps[:, :, 0:F_OUT],
                    in1=b2f_t,
                    op=ADD,
                )

            nc.sync.dma_start(out=out_r[t], in_=out_sb)

    nc.compile()
    return nc


_PROG = None


def _get_prog():
    global _PROG
    if _PROG is None:
        _PROG = build_program(NT)
    return _PROG


def _prepare_in_maps(x, edge_index, edge_attr, W1, b1, W2, b2):
    x = np.ascontiguousarray(np.asarray(x, dtype=np.float32))
    row = np.ascontiguousarray(np.asarray(edge_index, dtype=np.int64)[0])
    ea = np.asarray(edge_attr, dtype=np.float32)
    w1 = np.ascontiguousarray(np.asarray(W1, dtype=np.float32))
    w2p = np.zeros((HIDDEN, 2 * F_OUT), dtype=np.float32)
    w2p[:, :F_OUT] = np.asarray(W2, dtype=np.float32)
    b1c = np.ascontiguousarray(np.asarray(b1, dtype=np.float32).reshape(HIDDEN, 1))
    b2v = np.ascontiguousarray(np.asarray(b2, dtype=np.float32).reshape(F_OUT))
    x2 = x.reshape(N_NODES // 2, 2 * F_IN)

    in_maps = []
    for c in range(N_CORES):
        sl = slice(c * E_REAL, (c + 1) * E_REAL)
        row_pad = np.zeros((EPC,), dtype=np.int64)
        row_pad[:E_REAL] = row[sl]
        ea_pad = np.zeros((EPC, F_IN), dtype=np.float32)
        ea_pad[:E_REAL] = ea[sl]
        # dma_gather index wrap: sequence pos i = s*16 + p16 read from idxs[p16, s];
        # within a tile, dest position i = c*128 + p  (partition-minor edge order)
        hr = (row_pad >> 1).astype(np.int16)
        hidx = np.ascontiguousarray(
            np.tile(hr.reshape(NT, TILE_E // 16, 16).transpose(0, 2, 1), (1, 8, 1))
        ).reshape(NT * 128, TILE_E // 16)
        par = (row_pad & 1).astype(np.int8)
        par_r = np.ascontiguousarray(
            par.reshape(NT, KPT, 128).transpose(0, 2, 1)
        ).reshape(NT * 128, KPT)
        in_maps.append(
            {
                "x2": x2,
                "hidx": hidx,
                "par": par_r,
                "ea": ea_pad,
                "w1": w1,
                "w2p": w2p,
                "b1c": b1c,
                "b2": b2v,
            }
        )
    return in_maps


def run_spmd(inputs: dict, trace: bool = False, **spmd_kwargs):
    """Run the kernel on all 8 cores. Returns (output, BassKernelResults)."""
    in_maps = _prepare_in_maps(
        inputs["x"], inputs["edge_index"], inputs["edge_attr"],
        inputs["W1"], inputs["b1"], inputs["W2"], inputs["b2"],
    )
    nc = _get_prog()
    bres = run_bass_kernel_spmd(
        nc, in_maps, list(range(N_CORES)), trace=trace, **spmd_kwargs
    )
    res = bres.results
    # undo the partition-minor edge order: output row e is already in natural
    # order (out_d is indexed by e directly), so just trim the padding
    out = np.concatenate([res[c]["out"][:E_REAL] for c in range(N_CORES)], axis=0)
    return np.ascontiguousarray(out, dtype=np.float32), bres


def kernel(x, edge_index, edge_attr, u, batch, W1, b1, W2, b2):
    out, _ = run_spmd(
        {
            "x": x, "edge_index": edge_index, "edge_attr": edge_attr,
            "W1": W1, "b1": b1, "W2": W2, "b2": b2,
        }
    )
    return out



# revision 2
# speedup vs baseline: 1.6075x; 1.6075x over previous
"""Edge-parallel GNN message-passing MLP on 8 TRN2 NeuronCores.

Computation (per edge e): out[e] = relu(concat(x[row[e]], edge_attr[e]) @ W1 + b1) @ W2 + b2

Strategy (v2 — no per-edge DMA descriptors):
  The baseline dma_gather spent ~16 us per 2048-edge tile generating DMA
  descriptors in GpSimd Q7 software (~8 ns/edge serial) — 800 us of the
  970 us kernel. This version eliminates the gather entirely:

  * x lives RESIDENT in SBUF in fp16, row-major chunks of 128 rows
    ([128 parts, 392 chunks, 64 feats], 50 KB/partition), loaded once.
  * Edges are globally sorted by row and dealt round-robin to the 8 cores,
    so every core has the same per-chunk edge-count profile (+-1). Each
    128-row chunk c gets a static slot count s_c = roundup4(ceil(total_c/8))
    baked into the program (compiled per data profile, cached).
  * Per chunk, a one-hot selection matrix S[r, s] = (start_r <= s < end_r)
    is built in TWO DVE passes from per-partition run boundaries (edges in
    a chunk are row-sorted, so each row's slots are one contiguous run):
        ge = (iota >= start_r)          # tensor_scalar, per-partition scalar
        S  = (iota < end_r) * ge        # scalar_tensor_tensor
  * The "gather" is then a PE matmul: featsT_x = x_chunk.T @ S.
    feats = [x-part; edge_attr-part] feature-major, then the 2-layer MLP
    runs as plain matmuls (W1, W2 stationary), relu+b1 fused on the scalar
    engine, +b2 fused into the PSUM->SBUF drain on the vector engine.
  * Everything is fp16 on the wire (x, ea, weights, h1, out) with fp32
    PSUM accumulation; rel err ~5e-4.
  * edge_attr loads and output stores are batched per 16-chunk supergroup
    (2 DMA instructions per ~4100 slots, >4 KB descriptors, full rate).

Self-contained: shapes hardcoded for the 50000-node / 800000-edge problem.
"""

from contextlib import ExitStack

import numpy as np

import concourse.bacc as bacc_mod
import concourse.mybir as mybir
import concourse.tile as tile
from concourse.bass_utils import run_bass_kernel_spmd

N_CORES = 8
N_NODES = 50000
N_EDGES = 800000
F_IN = 64
HIDDEN = 128
F_OUT = 128

CHUNK_ROWS = 128
NCHUNK = (N_NODES + CHUNK_ROWS - 1) // CHUNK_ROWS  # 391
NPAD_ROWS = NCHUNK * CHUNK_ROWS                    # 50048
E_CORE = N_EDGES // N_CORES                        # 100000
SUPER = 16                                         # chunks per ea/out DMA batch

F32 = mybir.dt.float32
FP16 = mybir.dt.float16
I16 = mybir.dt.int16

RELU = mybir.ActivationFunctionType.Relu
ADD = mybir.AluOpType.add
MULT = mybir.AluOpType.mult
IS_GE = mybir.AluOpType.is_ge
IS_LT = mybir.AluOpType.is_lt


def _round_up(v, m):
    return (v + m - 1) // m * m


def build_program(s_prof: tuple[int, ...]):
    """Build the SPMD program for a per-chunk slot-count profile."""
    assert len(s_prof) == NCHUNK
    smax = max(s_prof)
    assert smax <= 512
    slot_base = np.concatenate([[0], np.cumsum(s_prof)]).astype(np.int64)
    total_slots = int(slot_base[-1])

    # supergroup boundaries (chunk indices) and widths
    groups = []
    for g0 in range(0, NCHUNK, SUPER):
        g1 = min(g0 + SUPER, NCHUNK)
        width = int(slot_base[g1] - slot_base[g0])
        groups.append((g0, g1, width))
    gwmax = max(w for _, _, w in groups)

    nc = bacc_mod.Bacc("TRN2")

    xr_d = nc.declare_dram_parameter("xr", [128, NCHUNK * F_IN], FP16, isOutput=False)
    meta_d = nc.declare_dram_parameter("meta", [128, NCHUNK * 2], F32, isOutput=False)
    iota_d = nc.declare_dram_parameter("iota", [128, smax], FP16, isOutput=False)
    ea_d = nc.declare_dram_parameter("eaT", [F_IN, max(total_slots, 1)], FP16, isOutput=False)
    w1_d = nc.declare_dram_parameter("w1", [2 * F_IN, HIDDEN], FP16, isOutput=False)
    w2_d = nc.declare_dram_parameter("w2", [HIDDEN, F_OUT], FP16, isOutput=False)
    b1_d = nc.declare_dram_parameter("b1c", [HIDDEN, 1], F32, isOutput=False)
    b2_d = nc.declare_dram_parameter("b2c", [F_OUT, 1], F32, isOutput=False)
    out_d = nc.declare_dram_parameter("outT", [F_OUT, max(total_slots, 1)], FP16, isOutput=True)

    with tile.TileContext(nc) as tc, ExitStack() as ctx:
        const = ctx.enter_context(tc.tile_pool(name="const", bufs=1))
        ge_p = ctx.enter_context(tc.tile_pool(name="ge", bufs=2))
        s_p = ctx.enter_context(tc.tile_pool(name="sel", bufs=2))
        feats_p = ctx.enter_context(tc.tile_pool(name="feats", bufs=2))
        h1_p = ctx.enter_context(tc.tile_pool(name="h1", bufs=2))
        osb_p = ctx.enter_context(tc.tile_pool(name="osb", bufs=2))
        expps_p = ctx.enter_context(tc.tile_pool(name="expps", bufs=2, space="PSUM"))
        l1ps_p = ctx.enter_context(tc.tile_pool(name="l1ps", bufs=2, space="PSUM"))
        l2ps_p = ctx.enter_context(tc.tile_pool(name="l2ps", bufs=2, space="PSUM"))

        # ---- resident constants ----
        xr_t = const.tile([128, NCHUNK * F_IN], FP16, tag="xr")
        nc.sync.dma_start(out=xr_t, in_=xr_d[:, :])
        meta_t = const.tile([128, NCHUNK * 2], F32, tag="meta")
        nc.sync.dma_start(out=meta_t, in_=meta_d[:, :])
        iota_t = const.tile([128, smax], FP16, tag="iota")
        nc.sync.dma_start(out=iota_t, in_=iota_d[:, :])
        w1_t = const.tile([128, HIDDEN], FP16, tag="w1")
        nc.sync.dma_start(out=w1_t, in_=w1_d[:, :])
        w2_t = const.tile([128, F_OUT], FP16, tag="w2")
        nc.sync.dma_start(out=w2_t, in_=w2_d[:, :])
        b1_t = const.tile([128, 1], F32, tag="b1")
        nc.sync.dma_start(out=b1_t, in_=b1_d[:, :])
        b2_t = const.tile([128, 1], F32, tag="b2")
        nc.sync.dma_start(out=b2_t, in_=b2_d[:, :])

        for g0, g1, gwidth in groups:
            gbase = int(slot_base[g0])
            feats = feats_p.tile([128, gwmax], FP16, tag="feats")
            osb = osb_p.tile([128, gwmax], FP16, tag="osb")

            # batched edge_attr load for the supergroup (feature-major)
            nc.sync.dma_start(
                out=feats[F_IN : 2 * F_IN, 0:gwidth],
                in_=ea_d[:, gbase : gbase + gwidth],
            )

            for c in range(g0, g1):
                sc = s_prof[c]
                if sc == 0:
                    continue
                off = int(slot_base[c]) - gbase

                # ---- one-hot S from per-row slot runs ----
                ge = ge_p.tile([128, smax], FP16, tag="ge")
                nc.vector.tensor_scalar(
                    out=ge[:, 0:sc],
                    in0=iota_t[:, 0:sc],
                    scalar1=meta_t[:, 2 * c : 2 * c + 1],
                    scalar2=None,
                    op0=IS_GE,
                )
                sel = s_p.tile([128, smax], FP16, tag="sel")
                nc.vector.scalar_tensor_tensor(
                    out=sel[:, 0:sc],
                    in0=iota_t[:, 0:sc],
                    scalar=meta_t[:, 2 * c + 1 : 2 * c + 2],
                    in1=ge[:, 0:sc],
                    op0=IS_LT,
                    op1=MULT,
                )

                # ---- expansion: featsT_x[64, sc] = x_chunk.T @ S ----
                eps = expps_p.tile([64, smax], F32, tag="eps", space="PSUM")
                nc.tensor.matmul(
                    out=eps[:, 0:sc],
                    lhsT=xr_t[:, c * F_IN : (c + 1) * F_IN],
                    rhs=sel[:, 0:sc],
                    start=True,
                    stop=True,
                )
                nc.vector.tensor_copy(
                    out=feats[0:F_IN, off : off + sc], in_=eps[:, 0:sc]
                )

                # ---- layer 1 + relu ----
                l1 = l1ps_p.tile([128, smax], F32, tag="l1", space="PSUM")
                nc.tensor.matmul(
                    out=l1[:, 0:sc],
                    lhsT=w1_t,
                    rhs=feats[:, off : off + sc],
                    start=True,
                    stop=True,
                )
                h1 = h1_p.tile([128, smax], FP16, tag="h1")
                nc.scalar.activation(
                    out=h1[:, 0:sc],
                    in_=l1[:, 0:sc],
                    func=RELU,
                    bias=b1_t,
                    scale=1.0,
                )

                # ---- layer 2 (+b2 fused into the PSUM drain) ----
                l2 = l2ps_p.tile([128, smax], F32, tag="l2", space="PSUM")
                nc.tensor.matmul(
                    out=l2[:, 0:sc],
                    lhsT=w2_t,
                    rhs=h1[:, 0:sc],
                    start=True,
                    stop=True,
                )
                nc.vector.tensor_scalar(
                    out=osb[:, off : off + sc],
                    in0=l2[:, 0:sc],
                    scalar1=b2_t,
                    scalar2=None,
                    op0=ADD,
                )

            nc.sync.dma_start(
                out=out_d[:, gbase : gbase + gwidth], in_=osb[:, 0:gwidth]
            )

    nc.compile()
    return nc


_PROG_CACHE: dict[tuple, object] = {}


def _get_prog(s_prof: tuple[int, ...]):
    prog = _PROG_CACHE.get(s_prof)
    if prog is None:
        prog = build_program(s_prof)
        _PROG_CACHE[s_prof] = prog
    return prog


def _prepare(x, edge_index, edge_attr, W1, b1, W2, b2):
    x = np.ascontiguousarray(np.asarray(x, dtype=np.float32))
    row = np.ascontiguousarray(np.asarray(edge_index, dtype=np.int64)[0])
    ea = np.asarray(edge_attr, dtype=np.float32)
    w1 = np.asarray(W1, dtype=np.float32)
    w2 = np.asarray(W2, dtype=np.float32)
    b1v = np.asarray(b1, dtype=np.float32).reshape(HIDDEN, 1)
    b2v = np.asarray(b2, dtype=np.float32).reshape(F_OUT, 1)

    # global row-sort; deal sorted edges round-robin to cores
    order_g = np.argsort(row, kind="stable")
    t_c = np.bincount(row >> 7, minlength=NCHUNK)
    s_prof = tuple(int(_round_up(-(-int(t) // N_CORES), 4)) for t in t_c)
    slot_base = np.concatenate([[0], np.cumsum(s_prof)]).astype(np.int64)
    total_slots = int(slot_base[-1])
    smax = max(s_prof)

    # x row-major chunks, fp16: xr[p, c*64+f] = x[128c+p, f]
    x_pad = np.zeros((NPAD_ROWS, F_IN), dtype=np.float16)
    x_pad[:N_NODES] = x.astype(np.float16)
    xr = np.ascontiguousarray(
        x_pad.reshape(NCHUNK, 128, F_IN).transpose(1, 0, 2)
    ).reshape(128, NCHUNK * F_IN)

    iota = np.broadcast_to(
        np.arange(smax, dtype=np.float16)[None, :], (128, smax)
    ).copy()

    in_maps = []
    slot_maps = []  # per core: original-edge-id -> slot
    for k in range(N_CORES):
        gsel = order_g[k::N_CORES]  # original edge ids, row-sorted
        rk = row[gsel]
        ck = rk >> 7
        m = np.bincount(ck, minlength=NCHUNK)
        assert (m <= np.asarray(s_prof)).all()
        cum0 = np.concatenate([[0], np.cumsum(m)]).astype(np.int64)
        local = np.arange(len(gsel), dtype=np.int64) - cum0[ck]
        slots = slot_base[ck] + local

        # run boundaries per padded row, local to the chunk's slot range
        bnd = np.searchsorted(rk, np.arange(NPAD_ROWS + 1))
        chunk_of_r = np.arange(NPAD_ROWS) >> 7
        start_l = bnd[:-1] - cum0[chunk_of_r]
        end_l = bnd[1:] - cum0[chunk_of_r]
        meta = np.empty((128, NCHUNK * 2), dtype=np.float32)
        sl = start_l.reshape(NCHUNK, 128).T  # [p, c]
        el = end_l.reshape(NCHUNK, 128).T
        meta[:, 0::2] = sl
        meta[:, 1::2] = el

        eaT = np.zeros((F_IN, total_slots), dtype=np.float16)
        eaT[:, slots] = ea[gsel].astype(np.float16).T

        in_maps.append(
            {
                "xr": xr,
                "meta": meta,
                "iota": iota,
                "eaT": eaT,
                "w1": w1.astype(np.float16),
                "w2": w2.astype(np.float16),
                "b1c": b1v,
                "b2c": b2v,
            }
        )
        slot_maps.append((gsel, slots))
    return s_prof, in_maps, slot_maps


def run_spmd(inputs: dict, trace: bool = False, **spmd_kwargs):
    """Run the kernel on all 8 cores. Returns (output, BassKernelResults)."""
    s_prof, in_maps, slot_maps = _prepare(
        inputs["x"], inputs["edge_index"], inputs["edge_attr"],
        inputs["W1"], inputs["b1"], inputs["W2"], inputs["b2"],
    )
    nc = _get_prog(s_prof)
    bres = run_bass_kernel_spmd(
        nc, in_maps, list(range(N_CORES)), trace=trace, **spmd_kwargs
    )
    out = np.empty((N_EDGES, F_OUT), dtype=np.float32)
    for k in range(N_CORES):
        gsel, slots = slot_maps[k]
        outT = bres.results[k]["outT"]  # [F_OUT, total_slots] fp16
        out[gsel] = outT[:, slots].T.astype(np.float32)
    return out, bres


def kernel(x, edge_index, edge_attr, u, batch, W1, b1, W2, b2):
    out, _ = run_spmd(
        {
            "x": x, "edge_index": edge_index, "edge_attr": edge_attr,
            "W1": W1, "b1": b1, "W2": W2, "b2": b2,
        }
    )
    return out


# revision 11
# speedup vs baseline: 2.5235x; 1.5699x over previous
"""Edge-parallel GNN message-passing MLP on 8 TRN2 NeuronCores.

Computation (per edge e): out[e] = relu(concat(x[row[e]], edge_attr[e]) @ W1 + b1) @ W2 + b2

Strategy (v2 — no per-edge DMA descriptors):
  The baseline dma_gather spent ~16 us per 2048-edge tile generating DMA
  descriptors in GpSimd Q7 software (~8 ns/edge serial) — 800 us of the
  970 us kernel. This version eliminates the gather entirely:

  * x lives RESIDENT in SBUF in fp16, row-major chunks of 128 rows
    ([128 parts, 392 chunks, 64 feats], 50 KB/partition), loaded once.
  * Edges are globally sorted by row and dealt round-robin to the 8 cores,
    so every core has the same per-chunk edge-count profile (+-1). Each
    128-row chunk c gets a static slot count s_c = roundup4(ceil(total_c/8))
    baked into the program (compiled per data profile, cached).
  * Per chunk, a one-hot selection matrix S[r, s] (1 where slot s holds an
    edge whose row is chunk-local row r) is built ON THE HOST in fp8e4m3
    (exact for 0/1) and streamed in per 16-chunk supergroup — the PE
    accepts an fp16-stationary x fp8-moving matmul, so S costs 1 byte/elem
    of DMA and zero vector-engine time.
  * The "gather" is then a PE matmul: featsT_x = x_chunk.T @ S.
    feats = [x-part; edge_attr-part] feature-major, then the 2-layer MLP
    runs as plain matmuls (W1, W2 stationary), relu+b1 fused on the scalar
    engine, +b2 fused into the PSUM->SBUF drain on the vector engine.
  * Everything is fp16 on the wire (x, ea, weights, h1, out) with fp32
    PSUM accumulation; rel err ~5e-4.
  * edge_attr loads and output stores are batched per 16-chunk supergroup
    (2 DMA instructions per ~4100 slots, >4 KB descriptors, full rate).

Self-contained: shapes hardcoded for the 50000-node / 800000-edge problem.
"""

from contextlib import ExitStack

import numpy as np

import concourse.bacc as bacc_mod
import concourse.mybir as mybir
import concourse.tile as tile
from concourse.bass_utils import run_bass_kernel_spmd

N_CORES = 8
N_NODES = 50000
N_EDGES = 800000
F_IN = 64
HIDDEN = 128
F_OUT = 128

CHUNK_ROWS = 128
NCHUNK = (N_NODES + CHUNK_ROWS - 1) // CHUNK_ROWS  # 391
NPAD_ROWS = NCHUNK * CHUNK_ROWS                    # 50048
E_CORE = N_EDGES // N_CORES                        # 100000
SUPER = 16                                         # chunks per ea/out DMA batch

F32 = mybir.dt.float32
FP16 = mybir.dt.float16
FP8 = mybir.dt.float8e4

RELU = mybir.ActivationFunctionType.Relu
IDENT = mybir.ActivationFunctionType.Identity


def _round_up(v, m):
    return (v + m - 1) // m * m


def build_program(s_prof: tuple[int, ...]):
    """Build the SPMD program for a per-chunk slot-count profile."""
    assert len(s_prof) == NCHUNK
    smax = max(s_prof)
    assert smax <= 512
    slot_base = np.concatenate([[0], np.cumsum(s_prof)]).astype(np.int64)
    total_slots = int(slot_base[-1])

    # supergroup boundaries (chunk indices) and widths
    groups = []
    for g0 in range(0, NCHUNK, SUPER):
        g1 = min(g0 + SUPER, NCHUNK)
        width = int(slot_base[g1] - slot_base[g0])
        groups.append((g0, g1, width))
    gwmax = max(w for _, _, w in groups)

    nc = bacc_mod.Bacc("TRN2")

    xr_d = nc.declare_dram_parameter("xr", [128, NCHUNK * F_IN], FP16, isOutput=False)
    s8_d = nc.declare_dram_parameter("s8", [128, max(total_slots, 1)], FP8, isOutput=False)
    ea_d = nc.declare_dram_parameter("eaT", [F_IN, max(total_slots, 1)], FP16, isOutput=False)
    w1_d = nc.declare_dram_parameter("w1", [2 * F_IN, HIDDEN], FP16, isOutput=False)
    w2_d = nc.declare_dram_parameter("w2", [HIDDEN, F_OUT], FP16, isOutput=False)
    b1_d = nc.declare_dram_parameter("b1c", [HIDDEN, 1], F32, isOutput=False)
    b2_d = nc.declare_dram_parameter("b2c", [F_OUT, 1], F32, isOutput=False)
    out_d = nc.declare_dram_parameter("outT", [F_OUT, max(total_slots, 1)], FP16, isOutput=True)

    with tile.TileContext(nc) as tc, ExitStack() as ctx:
        const = ctx.enter_context(tc.tile_pool(name="const", bufs=1))
        s8_p = ctx.enter_context(tc.tile_pool(name="s8", bufs=2))
        feats_p = ctx.enter_context(tc.tile_pool(name="feats", bufs=2))
        h1_p = ctx.enter_context(tc.tile_pool(name="h1", bufs=2))
        osb_p = ctx.enter_context(tc.tile_pool(name="osb", bufs=2))
        expps_p = ctx.enter_context(tc.tile_pool(name="expps", bufs=2, space="PSUM"))
        l1ps_p = ctx.enter_context(tc.tile_pool(name="l1ps", bufs=2, space="PSUM"))
        l2ps_p = ctx.enter_context(tc.tile_pool(name="l2ps", bufs=2, space="PSUM"))

        # ---- resident constants ----
        xr_t = const.tile([128, NCHUNK * F_IN], FP16, tag="xr")
        nc.sync.dma_start(out=xr_t, in_=xr_d[:, :])
        w1_t = const.tile([128, HIDDEN], FP16, tag="w1")
        nc.sync.dma_start(out=w1_t, in_=w1_d[:, :])
        w2_t = const.tile([128, F_OUT], FP16, tag="w2")
        nc.sync.dma_start(out=w2_t, in_=w2_d[:, :])
        b1_t = const.tile([128, 1], F32, tag="b1")
        nc.sync.dma_start(out=b1_t, in_=b1_d[:, :])
        b2_t = const.tile([128, 1], F32, tag="b2")
        nc.sync.dma_start(out=b2_t, in_=b2_d[:, :])

        for g0, g1, gwidth in groups:
            gbase = int(slot_base[g0])
            feats = feats_p.tile([128, gwmax], FP16, tag="feats")
            osb = osb_p.tile([128, gwmax], FP16, tag="osb")
            s8_t = s8_p.tile([128, gwmax], FP8, tag="s8")

            # batched supergroup loads: one-hot S (fp8) + edge_attr (fp16)
            nc.sync.dma_start(
                out=s8_t[:, 0:gwidth], in_=s8_d[:, gbase : gbase + gwidth]
            )
            nc.sync.dma_start(
                out=feats[F_IN : 2 * F_IN, 0:gwidth],
                in_=ea_d[:, gbase : gbase + gwidth],
            )

            for c in range(g0, g1):
                sc = s_prof[c]
                if sc == 0:
                    continue
                off = int(slot_base[c]) - gbase

                # ---- expansion: featsT_x[64, sc] = x_chunk.T @ S ----
                eps = expps_p.tile([64, smax], F32, tag="eps", space="PSUM")
                nc.tensor.matmul(
                    out=eps[:, 0:sc],
                    lhsT=xr_t[:, c * F_IN : (c + 1) * F_IN],
                    rhs=s8_t[:, off : off + sc],
                    start=True,
                    stop=True,
                )
                nc.vector.tensor_copy(
                    out=feats[0:F_IN, off : off + sc], in_=eps[:, 0:sc]
                )

                # ---- layer 1 + relu ----
                l1 = l1ps_p.tile([128, smax], F32, tag="l1", space="PSUM")
                nc.tensor.matmul(
                    out=l1[:, 0:sc],
                    lhsT=w1_t,
                    rhs=feats[:, off : off + sc],
                    start=True,
                    stop=True,
                )
                h1 = h1_p.tile([128, smax], FP16, tag="h1")
                nc.scalar.activation(
                    out=h1[:, 0:sc],
                    in_=l1[:, 0:sc],
                    func=RELU,
                    bias=b1_t,
                    scale=1.0,
                )

                # ---- layer 2 (+b2 fused into the PSUM drain) ----
                l2 = l2ps_p.tile([128, smax], F32, tag="l2", space="PSUM")
                nc.tensor.matmul(
                    out=l2[:, 0:sc],
                    lhsT=w2_t,
                    rhs=h1[:, 0:sc],
                    start=True,
                    stop=True,
                )
                # drain+b2: mostly DVE, every 3rd chunk on the scalar engine
                # to balance the two drain engines
                if c % 3 == 2:
                    nc.scalar.activation(
                        out=osb[:, off : off + sc],
                        in_=l2[:, 0:sc],
                        func=IDENT,
                        bias=b2_t,
                        scale=1.0,
                    )
                else:
                    nc.vector.tensor_scalar_add(
                        out=osb[:, off : off + sc],
                        in0=l2[:, 0:sc],
                        scalar1=b2_t,
                    )

            nc.sync.dma_start(
                out=out_d[:, gbase : gbase + gwidth], in_=osb[:, 0:gwidth]
            )

    nc.compile()
    return nc


_PROG_CACHE: dict[tuple, object] = {}


def _get_prog(s_prof: tuple[int, ...]):
    prog = _PROG_CACHE.get(s_prof)
    if prog is None:
        prog = build_program(s_prof)
        _PROG_CACHE[s_prof] = prog
    return prog


def _prepare(x, edge_index, edge_attr, W1, b1, W2, b2):
    x = np.ascontiguousarray(np.asarray(x, dtype=np.float32))
    row = np.ascontiguousarray(np.asarray(edge_index, dtype=np.int64)[0])
    ea = np.asarray(edge_attr, dtype=np.float32)
    w1 = np.asarray(W1, dtype=np.float32)
    w2 = np.asarray(W2, dtype=np.float32)
    b1v = np.asarray(b1, dtype=np.float32).reshape(HIDDEN, 1)
    b2v = np.asarray(b2, dtype=np.float32).reshape(F_OUT, 1)

    # global row-sort; deal sorted edges round-robin to cores
    order_g = np.argsort(row, kind="stable")
    t_c = np.bincount(row >> 7, minlength=NCHUNK)
    s_prof = tuple(int(_round_up(-(-int(t) // N_CORES), 4)) for t in t_c)
    slot_base = np.concatenate([[0], np.cumsum(s_prof)]).astype(np.int64)
    total_slots = int(slot_base[-1])
    smax = max(s_prof)

    # x row-major chunks, fp16: xr[p, c*64+f] = x[128c+p, f]
    x_pad = np.zeros((NPAD_ROWS, F_IN), dtype=np.float16)
    x_pad[:N_NODES] = x.astype(np.float16)
    xr = np.ascontiguousarray(
        x_pad.reshape(NCHUNK, 128, F_IN).transpose(1, 0, 2)
    ).reshape(128, NCHUNK * F_IN)

    import ml_dtypes

    in_maps = []
    slot_maps = []  # per core: original-edge-id -> slot
    for k in range(N_CORES):
        gsel = order_g[k::N_CORES]  # original edge ids, row-sorted
        rk = row[gsel]
        ck = rk >> 7
        m = np.bincount(ck, minlength=NCHUNK)
        assert (m <= np.asarray(s_prof)).all()
        cum0 = np.concatenate([[0], np.cumsum(m)]).astype(np.int64)
        local = np.arange(len(gsel), dtype=np.int64) - cum0[ck]
        slots = slot_base[ck] + local

        # one-hot selection matrix: S[row - 128*chunk, slot] = 1
        s8 = np.zeros((128, total_slots), dtype=ml_dtypes.float8_e4m3fn)
        s8[rk & 127, slots] = 1.0

        eaT = np.zeros((F_IN, total_slots), dtype=np.float16)
        eaT[:, slots] = ea[gsel].astype(np.float16).T

        in_maps.append(
            {
                "xr": xr,
                "s8": s8,
                "eaT": eaT,
                "w1": w1.astype(np.float16),
                "w2": w2.astype(np.float16),
                "b1c": b1v,
                "b2c": b2v,
            }
        )
        slot_maps.append((gsel, slots))
    return s_prof, in_maps, slot_maps


def run_spmd(inputs: dict, trace: bool = False, **spmd_kwargs):
    """Run the kernel on all 8 cores. Returns (output, BassKernelResults)."""
    s_prof, in_maps, slot_maps = _prepare(
        inputs["x"], inputs["edge_index"], inputs["edge_attr"],
        inputs["W1"], inputs["b1"], inputs["W2"], inputs["b2"],
    )
    nc = _get_prog(s_prof)
    bres = run_bass_kernel_spmd(
        nc, in_maps, list(range(N_CORES)), trace=trace, **spmd_kwargs
    )
    out = np.empty((N_EDGES, F_OUT), dtype=np.float32)
    for k in range(N_CORES):
        gsel, slots = slot_maps[k]
        outT = bres.results[k]["outT"]  # [F_OUT, total_slots] fp16
        out[gsel] = outT[:, slots].T.astype(np.float32)
    return out, bres


def kernel(x, edge_index, edge_attr, u, batch, W1, b1, W2, b2):
    out, _ = run_spmd(
        {
            "x": x, "edge_index": edge_index, "edge_attr": edge_attr,
            "W1": W1, "b1": b1, "W2": W2, "b2": b2,
        }
    )
    return out


# revision 15
# speedup vs baseline: 3.6712x; 1.4548x over previous
"""Edge-parallel GNN message-passing MLP on 8 TRN2 NeuronCores.

Computation (per edge e): out[e] = relu(concat(x[row[e]], edge_attr[e]) @ W1 + b1) @ W2 + b2

Strategy (v2 — no per-edge DMA descriptors):
  The baseline dma_gather spent ~16 us per 2048-edge tile generating DMA
  descriptors in GpSimd Q7 software (~8 ns/edge serial) — 800 us of the
  970 us kernel. This version eliminates the gather entirely:

  * x lives RESIDENT in SBUF in fp16, row-major chunks of 128 rows
    ([128 parts, 392 chunks, 64 feats], 50 KB/partition), loaded once.
  * Edges are globally sorted by row and dealt round-robin to the 8 cores,
    so every core has the same per-chunk edge-count profile (+-1). Each
    128-row chunk c gets a static slot count s_c = roundup4(ceil(total_c/8))
    baked into the program (compiled per data profile, cached).
  * Per chunk, a one-hot selection matrix S[r, s] (1 where slot s holds an
    edge whose row is chunk-local row r) is built ON THE HOST in fp8e4m3
    (exact for 0/1) and streamed in per 16-chunk supergroup — the PE
    accepts an fp16-stationary x fp8-moving matmul, so S costs 1 byte/elem
    of DMA and zero vector-engine time.
  * The "gather" is then a PE matmul: featsT_x = x_chunk.T @ S.
    feats = [x-part; edge_attr-part] feature-major, then the 2-layer MLP
    runs as plain matmuls (W1, W2 stationary), relu+b1 fused on the scalar
    engine, +b2 fused into the PSUM->SBUF drain on the vector engine.
  * Everything is fp16 on the wire (x, ea, weights, h1, out) with fp32
    PSUM accumulation; rel err ~5e-4.
  * edge_attr loads and output stores are batched per 16-chunk supergroup
    (2 DMA instructions per ~4100 slots, >4 KB descriptors, full rate).

Self-contained: shapes hardcoded for the 50000-node / 800000-edge problem.
"""

from contextlib import ExitStack

import numpy as np

import concourse.bacc as bacc_mod
import concourse.mybir as mybir
import concourse.tile as tile
from concourse.bass_utils import run_bass_kernel_spmd

N_CORES = 8
N_NODES = 50000
N_EDGES = 800000
F_IN = 64
HIDDEN = 128
F_OUT = 128

CHUNK_ROWS = 128
NCHUNK = 392                                       # chunks (incl. 1 pad chunk)
NPAD_ROWS = NCHUNK * CHUNK_ROWS                    # 50176
E_CORE = N_EDGES // N_CORES                        # 100000
SUPER = 16                                         # chunks per ea/out DMA batch
WIN = 512                                          # MLP window (1 PSUM bank)

F32 = mybir.dt.float32
FP16 = mybir.dt.float16
FP8 = mybir.dt.float8e4

RELU = mybir.ActivationFunctionType.Relu
IDENT = mybir.ActivationFunctionType.Identity


def _round_up(v, m):
    return (v + m - 1) // m * m


def build_program(s_prof: tuple[int, ...]):
    """Build the SPMD program for a per-chunk slot-count profile.

    s_prof must be pair-equal: s_prof[2i] == s_prof[2i+1] (each chunk pair
    shares one slot width so the pair's expansion PSUM drains in one copy).
    """
    assert len(s_prof) == NCHUNK
    smax = max(s_prof)
    assert smax <= 512
    slot_base = np.concatenate([[0], np.cumsum(s_prof)]).astype(np.int64)
    total_slots = int(slot_base[-1])

    # supergroup boundaries (chunk indices) and widths
    groups = []
    for g0 in range(0, NCHUNK, SUPER):
        g1 = min(g0 + SUPER, NCHUNK)
        width = int(slot_base[g1] - slot_base[g0])
        groups.append((g0, g1, width))
    gwmax = max(w for _, _, w in groups)

    nc = bacc_mod.Bacc("TRN2")

    xr_d = nc.declare_dram_parameter("xr", [128, NCHUNK * F_IN], FP16, isOutput=False)
    s8_d = nc.declare_dram_parameter("s8", [128, max(total_slots, 1)], FP8, isOutput=False)
    ea_d = nc.declare_dram_parameter("eaT", [F_IN, max(total_slots, 1)], FP16, isOutput=False)
    w1_d = nc.declare_dram_parameter("w1", [2 * F_IN, HIDDEN], FP16, isOutput=False)
    w2_d = nc.declare_dram_parameter("w2", [HIDDEN, F_OUT], FP16, isOutput=False)
    b1_d = nc.declare_dram_parameter("b1c", [HIDDEN, 1], F32, isOutput=False)
    b2_d = nc.declare_dram_parameter("b2c", [F_OUT, 1], F32, isOutput=False)
    out_d = nc.declare_dram_parameter("outT", [F_OUT, max(total_slots, 1)], FP16, isOutput=True)

    with tile.TileContext(nc) as tc, ExitStack() as ctx:
        const = ctx.enter_context(tc.tile_pool(name="const", bufs=1))
        s8_p = ctx.enter_context(tc.tile_pool(name="s8", bufs=2))
        feats_p = ctx.enter_context(tc.tile_pool(name="feats", bufs=2))
        h1_p = ctx.enter_context(tc.tile_pool(name="h1", bufs=2))
        osb_p = ctx.enter_context(tc.tile_pool(name="osb", bufs=2))
        expps_p = ctx.enter_context(tc.tile_pool(name="expps", bufs=2, space="PSUM"))
        l1ps_p = ctx.enter_context(tc.tile_pool(name="l1ps", bufs=2, space="PSUM"))
        l2ps_p = ctx.enter_context(tc.tile_pool(name="l2ps", bufs=2, space="PSUM"))

        # ---- resident constants ----
        xr_t = const.tile([128, NCHUNK * F_IN], FP16, tag="xr")
        nc.sync.dma_start(out=xr_t, in_=xr_d[:, :])
        w1_t = const.tile([128, HIDDEN], FP16, tag="w1")
        nc.sync.dma_start(out=w1_t, in_=w1_d[:, :])
        w2_t = const.tile([128, F_OUT], FP16, tag="w2")
        nc.sync.dma_start(out=w2_t, in_=w2_d[:, :])
        b1_t = const.tile([128, 1], F32, tag="b1")
        nc.sync.dma_start(out=b1_t, in_=b1_d[:, :])
        b2_t = const.tile([128, 1], F32, tag="b2")
        nc.sync.dma_start(out=b2_t, in_=b2_d[:, :])

        w_idx = 0
        for g0, g1, gwidth in groups:
            gbase = int(slot_base[g0])
            feats = feats_p.tile([128, gwmax], FP16, tag="feats")
            osb = osb_p.tile([128, gwmax], FP16, tag="osb")
            s8_t = s8_p.tile([128, gwmax], FP8, tag="s8")

            # batched supergroup loads: one-hot S (fp8) + edge_attr (fp16)
            nc.sync.dma_start(
                out=s8_t[:, 0:gwidth], in_=s8_d[:, gbase : gbase + gwidth]
            )
            nc.sync.dma_start(
                out=feats[F_IN : 2 * F_IN, 0:gwidth],
                in_=ea_d[:, gbase : gbase + gwidth],
            )

            # ---- expansion per chunk pair: featsT_x = x_chunk.T @ S ----
            for c0 in range(g0, g1, 2):
                sp = s_prof[c0]
                if sp == 0:
                    continue
                off = int(slot_base[c0]) - gbase
                # pair PSUM: [64, 2, WIN] — each half bank-aligned
                eps = expps_p.tile([64, 2, WIN], F32, tag="eps", space="PSUM")
                for j in (0, 1):
                    c = c0 + j
                    nc.tensor.matmul(
                        out=eps[:, j, 0:sp],
                        lhsT=xr_t[:, c * F_IN : (c + 1) * F_IN],
                        rhs=s8_t[:, off + j * sp : off + (j + 1) * sp],
                        start=True,
                        stop=True,
                    )
                # one drain for the pair
                nc.vector.tensor_copy(
                    out=feats[0:F_IN, off : off + 2 * sp].rearrange(
                        "f (j s) -> f j s", j=2
                    ),
                    in_=eps[:, :, 0:sp],
                )

            # ---- MLP on 512-slot windows over the supergroup ----
            for w in range(0, gwidth, WIN):
                wl = min(WIN, gwidth - w)
                l1 = l1ps_p.tile([128, WIN], F32, tag="l1", space="PSUM")
                nc.tensor.matmul(
                    out=l1[:, 0:wl],
                    lhsT=w1_t,
                    rhs=feats[:, w : w + wl],
                    start=True,
                    stop=True,
                )
                h1 = h1_p.tile([128, WIN], FP16, tag="h1")
                nc.scalar.activation(
                    out=h1[:, 0:wl],
                    in_=l1[:, 0:wl],
                    func=RELU,
                    bias=b1_t,
                    scale=1.0,
                )
                l2 = l2ps_p.tile([128, WIN], F32, tag="l2", space="PSUM")
                nc.tensor.matmul(
                    out=l2[:, 0:wl],
                    lhsT=w2_t,
                    rhs=h1[:, 0:wl],
                    start=True,
                    stop=True,
                )
                # drain+b2 split ~60/40 DVE/scalar to balance engines
                if w_idx % 5 < 2:
                    nc.scalar.activation(
                        out=osb[:, w : w + wl],
                        in_=l2[:, 0:wl],
                        func=IDENT,
                        bias=b2_t,
                        scale=1.0,
                    )
                else:
                    nc.vector.tensor_scalar_add(
                        out=osb[:, w : w + wl],
                        in0=l2[:, 0:wl],
                        scalar1=b2_t,
                    )
                w_idx += 1

            nc.sync.dma_start(
                out=out_d[:, gbase : gbase + gwidth], in_=osb[:, 0:gwidth]
            )

    nc.compile()
    return nc


_PROG_CACHE: dict[tuple, object] = {}


def _get_prog(s_prof: tuple[int, ...]):
    prog = _PROG_CACHE.get(s_prof)
    if prog is None:
        prog = build_program(s_prof)
        _PROG_CACHE[s_prof] = prog
    return prog


def _prepare(x, edge_index, edge_attr, W1, b1, W2, b2):
    x = np.ascontiguousarray(np.asarray(x, dtype=np.float32))
    row = np.ascontiguousarray(np.asarray(edge_index, dtype=np.int64)[0])
    ea = np.asarray(edge_attr, dtype=np.float32)
    w1 = np.asarray(W1, dtype=np.float32)
    w2 = np.asarray(W2, dtype=np.float32)
    b1v = np.asarray(b1, dtype=np.float32).reshape(HIDDEN, 1)
    b2v = np.asarray(b2, dtype=np.float32).reshape(F_OUT, 1)

    # global row-sort; deal sorted edges round-robin to cores
    order_g = np.argsort(row, kind="stable")
    t_c = np.bincount(row >> 7, minlength=NCHUNK)
    s_raw = [-(-int(t) // N_CORES) for t in t_c]
    # pair-equal slot widths (chunk pairs share one expansion-PSUM drain)
    s_prof = []
    for i in range(0, NCHUNK, 2):
        sp = _round_up(max(s_raw[i], s_raw[i + 1]), 4)
        s_prof += [sp, sp]
    s_prof = tuple(s_prof)
    slot_base = np.concatenate([[0], np.cumsum(s_prof)]).astype(np.int64)
    total_slots = int(slot_base[-1])
    smax = max(s_prof)

    # x row-major chunks, fp16: xr[p, c*64+f] = x[128c+p, f]
    x_pad = np.zeros((NPAD_ROWS, F_IN), dtype=np.float16)
    x_pad[:N_NODES] = x.astype(np.float16)
    xr = np.ascontiguousarray(
        x_pad.reshape(NCHUNK, 128, F_IN).transpose(1, 0, 2)
    ).reshape(128, NCHUNK * F_IN)

    import ml_dtypes

    in_maps = []
    slot_maps = []  # per core: original-edge-id -> slot
    for k in range(N_CORES):
        gsel = order_g[k::N_CORES]  # original edge ids, row-sorted
        rk = row[gsel]
        ck = rk >> 7
        m = np.bincount(ck, minlength=NCHUNK)
        assert (m <= np.asarray(s_prof)).all()
        cum0 = np.concatenate([[0], np.cumsum(m)]).astype(np.int64)
        local = np.arange(len(gsel), dtype=np.int64) - cum0[ck]
        slots = slot_base[ck] + local

        # one-hot selection matrix: S[row - 128*chunk, slot] = 1
        s8 = np.zeros((128, total_slots), dtype=ml_dtypes.float8_e4m3fn)
        s8[rk & 127, slots] = 1.0

        eaT = np.zeros((F_IN, total_slots), dtype=np.float16)
        eaT[:, slots] = ea[gsel].astype(np.float16).T

        in_maps.append(
            {
                "xr": xr,
                "s8": s8,
                "eaT": eaT,
                "w1": w1.astype(np.float16),
                "w2": w2.astype(np.float16),
                "b1c": b1v,
                "b2c": b2v,
            }
        )
        slot_maps.append((gsel, slots))
    return s_prof, in_maps, slot_maps


def run_spmd(inputs: dict, trace: bool = False, **spmd_kwargs):
    """Run the kernel on all 8 cores. Returns (output, BassKernelResults)."""
    s_prof, in_maps, slot_maps = _prepare(
        inputs["x"], inputs["edge_index"], inputs["edge_attr"],
        inputs["W1"], inputs["b1"], inputs["W2"], inputs["b2"],
    )
    nc = _get_prog(s_prof)
    bres = run_bass_kernel_spmd(
        nc, in_maps, list(range(N_CORES)), trace=trace, **spmd_kwargs
    )
    out = np.empty((N_EDGES, F_OUT), dtype=np.float32)
    for k in range(N_CORES):
        gsel, slots = slot_maps[k]
        outT = bres.results[k]["outT"]  # [F_OUT, total_slots] fp16
        out[gsel] = outT[:, slots].T.astype(np.float32)
    return out, bres


def kernel(x, edge_index, edge_attr, u, batch, W1, b1, W2, b2):
    out, _ = run_spmd(
        {
            "x": x, "edge_index": edge_index, "edge_attr": edge_attr,
            "W1": W1, "b1": b1, "W2": W2, "b2": b2,
        }
    )
    return out
